# revision 18
# baseline (speedup 1.0000x reference)
"""Trainium2 Bass kernel for masked (LSH/GNN) attention layer.

Computes, for full inputs (N=8192, F=512, D=64):
    kh = input @ kW; vh = input @ vW
    K  = (kh @ kh.T) / sqrt(D)
    scores = where(adj > 0, K, -9e15)
    att = softmax(scores, axis=1)
    out = elu(att @ vh)

Sharding: rows of the attention matrix across 8 NeuronCores (row-parallel).

The dominant cost in this environment is host->device staging over the
axon tunnel (~30-40 MB/s), so the input marshalling is laid out to move
as few bytes as possible per call (~11MB vs the baseline's ~400MB):
  - the tiny projections kh/vh ([N,64] each, 0.5% of the FLOPs) are
    computed on host from input/kW/vW, so the 16MB input matrix is never
    shipped (the baseline shipped it replicated to all 8 cores = 128MB);
  - kh/vh are shipped as fp16 *shards* (256KB/core each) and assembled
    on-device with an HBM-HBM AllGather across the 8 cores instead of
    being replicated; the N^2 attention math stays on-device;
  - adj is bit-packed on host (np.packbits) from 256MB int32 to 8MB of
    u8 (1MB/core), and unpacked on-device with shift/and tensor_scalar
    ops into the additive -9e15 mask;
  - output returns as fp16 (E<=1 and |h'| are well inside fp16 range;
    overall rel err ~1e-3 vs the 2e-2 gate);
  - the persistent jax compilation cache turns the runner's per-call
    re-jit of the identical HLO into a disk hit.
Device pipeline per 128-row block (row shard R=1024 per core):
    DMA:        adjp [128, N/8] u8 (packed adjacency rows)
    DVE:        unpack bits -> {0,1} u8 [128, N] (8 strided tensor_scalar)
    gpsimd:     madj = bits*9e15 - 9e15   in {0, -9e15}
    PE mm1:     S_raw = kh_own @ khT       (K=64 contraction, fp16)
    DVE:        sm = S_raw + madj ; full-row max (keeps E <= 1 for fp16)
    ACT:        E = exp((sm - rowmax)/sqrt(d)), accum_out -> densum
    PE:         transpose E 128x128 tiles -> PSUM, cast-copy f16 -> SBUF
    PE mm2:     h += E_tile @ vh_tile  (accumulate over 64 column tiles)
    epilogue:   h/densum, ELU, DMA out (fp16)
"""

import os
import sys
import math

sys.path.insert(0, "/opt/trn_rl_repo")

import numpy as np

# The runner constructs a fresh jax.jit closure per call, so the identical
# HLO is re-lowered and re-compiled every invocation; the persistent cache
# turns those recompiles into disk hits.
try:
    import jax
    jax.config.update("jax_compilation_cache_dir", "/tmp/jax_comp_cache")
    jax.config.update("jax_persistent_cache_min_compile_time_secs", 0)
    jax.config.update("jax_persistent_cache_min_entry_size_bytes", -1)
except Exception:
    pass

import concourse.bass as bass
import concourse.mybir as mybir
import concourse.tile as tile
from concourse import bacc
from concourse import bass_utils
from concourse.masks import make_identity

F32 = mybir.dt.float32
F16 = mybir.dt.float16
U8 = mybir.dt.uint8
AF = mybir.ActivationFunctionType
ALU = mybir.AluOpType

NEG_BIG = 9.0e15  # matches reference NEG_INF magnitude


def build_kernel(n=8192, r_shard=1024, d=64, num_devices=8, use_cc=False,
                 quant_out=False, debug_blk=None):
    """Build the SPMD-uniform Bass program for one core.

    use_cc=False: khT [d,n] and vh [n,d] arrive replicated per core.
    use_cc=True:  only the per-core shards khT_own [d,R] / vh_own [R,d]
                  arrive; full khT/vh are assembled on-device with an
                  HBM-HBM AllGather across the 8 cores.
    """
    P = 128
    assert n % 1024 == 0 and r_shard % P == 0
    n_rblk = r_shard // P          # 128-row blocks per core
    n_ctile = n // P               # 128-col tiles
    wpk = n // 8                   # packed bytes per row

    nc = bacc.Bacc("TRN2", target_bir_lowering=False, debug=False,
                   enable_asserts=False, num_devices=num_devices)

    khT_own_d = nc.dram_tensor("khT_own", [d, r_shard], F16,
                               kind="ExternalInput").ap()
    adjp_d = nc.dram_tensor("adjp", [r_shard, wpk], U8,
                            kind="ExternalInput").ap()
    if quant_out:
        out_d = nc.dram_tensor("out", [r_shard, d], U8,
                               kind="ExternalOutput").ap()
        oscale_d = nc.dram_tensor("oscale", [r_shard, 1], F32,
                                  kind="ExternalOutput").ap()
    else:
        out_d = nc.dram_tensor("out", [r_shard, d], F16,
                               kind="ExternalOutput").ap()
    if debug_blk is not None:
        dbg_sm_d = nc.dram_tensor("dbg_sm", [P, 1024], F32,
                                  kind="ExternalOutput").ap()
        dbg_exp_d = nc.dram_tensor("dbg_exp", [P, 1024], F32,
                                   kind="ExternalOutput").ap()
        dbg_stats_d = nc.dram_tensor("dbg_stats", [P, 4], F32,
                                     kind="ExternalOutput").ap()
        dbg_et_d = nc.dram_tensor("dbg_et", [P, 512], F16,
                                  kind="ExternalOutput").ap()
        dbg_h_d = nc.dram_tensor("dbg_h", [P, d], F32,
                                 kind="ExternalOutput").ap()
    if use_cc:
        vh_own_d = nc.dram_tensor("vh_own", [r_shard, d], F16,
                                  kind="ExternalInput").ap()
    else:
        khT_d = nc.dram_tensor("khT", [d, n], F16, kind="ExternalInput").ap()
        vh_d = nc.dram_tensor("vh", [n, d], F16, kind="ExternalInput").ap()

    with tile.TileContext(nc) as tc:
        with tc.tile_pool(name="const", bufs=1) as cpool:
            ident = cpool.tile([P, P], F32, tag="ident")
            khT = cpool.tile([d, n], F16, tag="khT")
            khT_own = cpool.tile([d, r_shard], F16, tag="khTo")
            vh_sb = cpool.tile([P, n_ctile * d], F16, tag="vh")
            make_identity(nc, ident[:])

            nc.sync.dma_start(khT_own[:], khT_own_d[:, :])
            if use_cc:
                ncores = num_devices
                with tc.tile_pool(name="dram", bufs=1, space="DRAM") as dram:
                    kh_in = dram.tile([d, r_shard], F16, tag="khin")
                    kh_out = dram.tile([ncores * d, r_shard], F16, tag="khout")
                    vh_in = dram.tile([r_shard, d], F16, tag="vhin")
                    vh_out = dram.tile([ncores * r_shard, d], F16, tag="vhout")
                    nc.gpsimd.dma_start(kh_in[:], khT_own_d[:, :])
                    nc.gpsimd.dma_start(vh_in[:], vh_own_d[:, :])
                    groups = [list(range(ncores))]
                    nc.gpsimd.collective_compute(
                        "AllGather", ALU.bypass, replica_groups=groups,
                        ins=[kh_in.opt()], outs=[kh_out.opt()])
                    nc.gpsimd.collective_compute(
                        "AllGather", ALU.bypass, replica_groups=groups,
                        ins=[vh_in.opt()], outs=[vh_out.opt()])
                    for c in range(ncores):
                        nc.sync.dma_start(
                            khT[:, c * r_shard:(c + 1) * r_shard],
                            kh_out[c * d:(c + 1) * d, :])
                    for j in range(n_ctile):
                        nc.sync.dma_start(
                            vh_sb[:, j * d:(j + 1) * d],
                            vh_out[j * P:(j + 1) * P, :])
            else:
                for c in range(4):
                    w = n // 4
                    nc.sync.dma_start(khT[:, c * w:(c + 1) * w],
                                      khT_d[:, c * w:(c + 1) * w])
                for j in range(n_ctile):
                    nc.sync.dma_start(vh_sb[:, j * d:(j + 1) * d],
                                      vh_d[j * P:(j + 1) * P, :])

            # ---------------- main loop over 128-row blocks --------------
            with tc.tile_pool(name="adjp", bufs=3) as adjpp, \
                 tc.tile_pool(name="bits", bufs=2) as bitsp, \
                 tc.tile_pool(name="madj", bufs=2) as madjp, \
                 tc.tile_pool(name="sm", bufs=2) as smp, \
                 tc.tile_pool(name="et", bufs=3) as etp, \
                 tc.tile_pool(name="small", bufs=2) as smallp, \
                 tc.tile_pool(name="sp", bufs=2, space="PSUM") as spp, \
                 tc.tile_pool(name="tp", bufs=2, space="PSUM") as tpp, \
                 tc.tile_pool(name="hp", bufs=2, space="PSUM") as hpp:

                c_chunk = 1024
                for b in range(n_rblk):
                    r0 = b * P
                    sm = smp.tile([P, n], F32, tag="sm")

                    adjp_t = adjpp.tile([P, wpk], U8, tag="adjp")
                    nc.sync.dma_start(adjp_t[:], adjp_d[r0:r0 + P, :])
                    bits = bitsp.tile([P, n], U8, tag="bits")
                    for k in range(8):
                        nc.vector.tensor_scalar(
                            bits[:, k::8], adjp_t[:], 7 - k, 1,
                            ALU.logical_shift_right, ALU.bitwise_and)

                    for i in range(n // c_chunk):
                        c0 = i * c_chunk
                        madj_t = madjp.tile([P, c_chunk], F32, tag="madj")
                        nc.gpsimd.tensor_scalar(
                            madj_t[:], bits[:, c0:c0 + c_chunk],
                            NEG_BIG, NEG_BIG, ALU.mult, ALU.subtract)
                        sp = spp.tile([P, c_chunk], F32, tag="sp")
                        for half in range(c_chunk // 512):
                            h0 = half * 512
                            nc.tensor.matmul(
                                sp[:, h0:h0 + 512],
                                khT_own[:, r0:r0 + P],
                                khT[:, c0 + h0:c0 + h0 + 512],
                                start=True, stop=True)
                        nc.vector.tensor_tensor(
                            sm[:, c0:c0 + c_chunk], sp[:], madj_t[:], ALU.add)

                    if debug_blk == b:
                        nc.sync.dma_start(dbg_sm_d[:, :], sm[:, 0:1024])
                    # full-row max: E = exp(s - max) <= 1 keeps the f16
                    # E-tiles within range (sampled max can undershoot by
                    # >80 when the diagonal falls outside the sample window)
                    nsamp = n
                    chmax = smallp.tile([P, 1], F32, tag="chmax")
                    nc.vector.tensor_reduce(chmax[:], sm[:, 0:nsamp],
                                            mybir.AxisListType.X, ALU.max)
                    negmax = smallp.tile([P, 1], F32, tag="negmax")
                    nc.vector.tensor_scalar_mul(
                        negmax[:], chmax[:], -1.0 / math.sqrt(d))
                    densum = smallp.tile([P, 1], F32, tag="densum")
                    nc.scalar.activation(sm[:], sm[:], AF.Exp,
                                         bias=negmax[:],
                                         scale=1.0 / math.sqrt(d),
                                         accum_out=densum[:])
                    recipd = smallp.tile([P, 1], F32, tag="recipd")
                    nc.vector.reciprocal(recipd[:], densum[:])
                    if debug_blk == b:
                        nc.sync.dma_start(dbg_exp_d[:, :], sm[:, 0:1024])
                        stats = smallp.tile([P, 4], F32, tag="stats")
                        nc.vector.tensor_copy(stats[:, 0:1], chmax[:])
                        nc.vector.tensor_copy(stats[:, 1:2], negmax[:])
                        nc.vector.tensor_copy(stats[:, 2:3], densum[:])
                        nc.vector.tensor_copy(stats[:, 3:4], recipd[:])
                        nc.sync.dma_start(dbg_stats_d[:, :], stats[:])

                    hpt = hpp.tile([P, d], F32, tag="hp")
                    for j4 in range(n_ctile // 4):
                        tp = tpp.tile([P, 512], F32, tag="tp")
                        for q in range(4):
                            j = j4 * 4 + q
                            nc.tensor.transpose(
                                tp[:, q * P:(q + 1) * P],
                                sm[:, j * P:(j + 1) * P], ident[:])
                        et = etp.tile([P, 512], F16, tag="et")
                        if j4 % 2 == 1:
                            nc.vector.tensor_copy(et[:], tp[:])
                        else:
                            nc.scalar.copy(et[:], tp[:])
                        if debug_blk == b and j4 == 0:
                            nc.sync.dma_start(dbg_et_d[:, :], et[:])
                        for q in range(4):
                            j = j4 * 4 + q
                            nc.tensor.matmul(
                                hpt[:], et[:, q * P:(q + 1) * P],
                                vh_sb[:, j * d:(j + 1) * d],
                                start=(j == 0), stop=(j == n_ctile - 1))

                    h_sb = smallp.tile([P, d], F32, tag="h")
                    nc.vector.tensor_scalar(h_sb[:], hpt[:], recipd[:], None,
                                            ALU.mult)
                    if debug_blk == b:
                        nc.sync.dma_start(dbg_h_d[:, :], h_sb[:])
                    hneg = smallp.tile([P, d], F32, tag="hneg")
                    nc.vector.tensor_scalar(hneg[:], h_sb[:], 0.0, None, ALU.min)
                    hpos = smallp.tile([P, d], F32, tag="hpos")
                    nc.vector.tensor_scalar(hpos[:], h_sb[:], 0.0, None, ALU.max)
                    nc.scalar.activation(hneg[:], hneg[:], AF.Exp)
                    nc.vector.tensor_scalar(hneg[:], hneg[:], -1.0, None, ALU.add)
                    if quant_out:
                        # symmetric u8 quantization with per-row absmax scale:
                        # q = out*127/absmax + 128.5 (u8 cast), decoded on host
                        # as (q-128)*absmax/127; <=1 LSB = absmax/127 error
                        out_f = smallp.tile([P, d], F32, tag="outf")
                        nc.vector.tensor_tensor(out_f[:], hneg[:], hpos[:],
                                                ALU.add)
                        omax = smallp.tile([P, 1], F32, tag="omax")
                        nc.vector.tensor_reduce(omax[:], out_f[:],
                                                mybir.AxisListType.X, ALU.max)
                        omin = smallp.tile([P, 1], F32, tag="omin")
                        nc.vector.tensor_reduce(omin[:], out_f[:],
                                                mybir.AxisListType.X, ALU.min)
                        negmin = smallp.tile([P, 1], F32, tag="negmin")
                        nc.vector.tensor_scalar_mul(negmin[:], omin[:], -1.0)
                        oabs = smallp.tile([P, 1], F32, tag="oabs")
                        nc.vector.tensor_tensor(oabs[:], omax[:], negmin[:],
                                                ALU.max)
                        nc.vector.tensor_scalar_max(oabs[:], oabs[:], 1e-20)
                        orcp = smallp.tile([P, 1], F32, tag="orcp")
                        nc.vector.reciprocal(orcp[:], oabs[:])
                        qs = smallp.tile([P, 1], F32, tag="qs")
                        nc.vector.tensor_scalar_mul(qs[:], orcp[:], 127.0)
                        qt = smallp.tile([P, d], U8, tag="qt")
                        nc.gpsimd.tensor_scalar(qt[:], out_f[:], qs[:], 128.5,
                                                ALU.mult, ALU.add)
                        nc.sync.dma_start(out_d[r0:r0 + P, :], qt[:])
                        nc.sync.dma_start(oscale_d[r0:r0 + P, :], oabs[:])
                    else:
                        out_t = smallp.tile([P, d], F16, tag="outt")
                        nc.vector.tensor_tensor(out_t[:], hneg[:], hpos[:],
                                                ALU.add)
                        nc.sync.dma_start(out_d[r0:r0 + P, :], out_t[:])

    nc.compile()
    return nc


_NC_CACHE = {}

USE_CC = True
QUANT_OUT = True


def _get_nc(key=(8192, 1024, 64, 8, USE_CC, QUANT_OUT)):
    if key not in _NC_CACHE:
        _NC_CACHE[key] = build_kernel(*key)
    return _NC_CACHE[key]


def gather_out(res, ncores=8, quant_out=None):
    """Assemble the full [N, 64] f32 output from per-core results."""
    if quant_out is None:
        quant_out = QUANT_OUT
    out = np.concatenate([res.results[k]["out"] for k in range(ncores)],
                         axis=0).astype(np.float32)
    if quant_out:
        s = np.concatenate([res.results[k]["oscale"] for k in range(ncores)],
                           axis=0).astype(np.float32)
        out = (out - 128.0) * (s / 127.0)
    return out


def prep_inputs(input, adj, kW, vW, ncores=8, use_cc=USE_CC):
    """Host-side marshalling: project kh/vh, bit-pack adj, slice shards."""
    N, D = 8192, 64
    RS = N // ncores
    input = np.asarray(input, np.float32)
    kW = np.asarray(kW, np.float32)
    vW = np.asarray(vW, np.float32)
    kh = input @ kW                       # [N, D]
    vh = np.ascontiguousarray((input @ vW).astype(np.float16))  # [N, D]
    khT = np.ascontiguousarray(kh.T.astype(np.float16))  # [D, N]
    adjp = np.packbits(np.asarray(adj) > 0, axis=1)  # [N, N/8] u8

    in_maps = []
    for c in range(ncores):
        m = {
            "khT_own": np.ascontiguousarray(khT[:, c * RS:(c + 1) * RS]),
            "adjp": adjp[c * RS:(c + 1) * RS, :],
        }
        if use_cc:
            m["vh_own"] = vh[c * RS:(c + 1) * RS, :]
        else:
            m["khT"] = khT
            m["vh"] = vh
        in_maps.append(m)
    return in_maps


def kernel(input, adj, kW, vW):
    N, D, NCORES = 8192, 64, 8
    RS = N // NCORES
    assert np.asarray(input).shape == (N, 512) and np.asarray(adj).shape == (N, N)

    nc = _get_nc((N, RS, D, NCORES, USE_CC, QUANT_OUT))
    in_maps = prep_inputs(input, adj, kW, vW, NCORES, USE_CC)
    res = bass_utils.run_bass_kernel_spmd(nc, in_maps,
                                          core_ids=list(range(NCORES)))
    return gather_out(res, NCORES)


if __name__ == "__main__":
    rng = np.random.default_rng(0)
    x = rng.standard_normal((8192, 512), dtype=np.float32)
    a = (rng.random((8192, 8192)) < 0.5).astype(np.int32)
    kw = rng.uniform(-0.1, 0.1, (512, 64)).astype(np.float32)
    vw = rng.uniform(-0.1, 0.1, (512, 64)).astype(np.float32)
    o = kernel(input=x, adj=a, kW=kw, vW=vw)
    print(o.shape, o.dtype, np.abs(o).max())


# revision 19
# speedup vs baseline: 1.5459x; 1.5459x over previous
"""Trainium2 Bass kernel for masked (LSH/GNN) attention layer.

Computes, for full inputs (N=8192, F=512, D=64):
    kh = input @ kW; vh = input @ vW
    K  = (kh @ kh.T) / sqrt(D)
    scores = where(adj > 0, K, -9e15)
    att = softmax(scores, axis=1)
    out = elu(att @ vh)

Sharding: rows of the attention matrix across 8 NeuronCores (row-parallel).

The dominant cost in this environment is host->device staging over the
axon tunnel (~30-40 MB/s), so the input marshalling is laid out to move
as few bytes as possible per call (~11MB vs the baseline's ~400MB):
  - the tiny projections kh/vh ([N,64] each, 0.5% of the FLOPs) are
    computed on host from input/kW/vW, so the 16MB input matrix is never
    shipped (the baseline shipped it replicated to all 8 cores = 128MB);
  - kh/vh are shipped as fp16 *shards* (256KB/core each) and assembled
    on-device with an HBM-HBM AllGather across the 8 cores instead of
    being replicated; the N^2 attention math stays on-device;
  - adj is bit-packed on host (np.packbits) from 256MB int32 to 8MB of
    u8 (1MB/core), and unpacked on-device with shift/and tensor_scalar
    ops into the additive -9e15 mask;
  - output returns as fp16 (E<=1 and |h'| are well inside fp16 range;
    overall rel err ~1e-3 vs the 2e-2 gate);
  - the persistent jax compilation cache turns the runner's per-call
    re-jit of the identical HLO into a disk hit.
Device pipeline per 128-row block (row shard R=1024 per core):
    DMA:        adjp [128, N/8] u8 (packed adjacency rows)
    DVE:        unpack bits -> {0,1} u8 [128, N] (8 strided tensor_scalar)
    gpsimd:     madj = bits*9e15 - 9e15   in {0, -9e15}
    PE mm1:     S_raw = kh_own @ khT       (K=64 contraction, fp16)
    DVE:        sm = S_raw + madj ; full-row max (keeps E <= 1 for fp16)
    ACT:        E = exp((sm - rowmax)/sqrt(d)), accum_out -> densum
    PE:         transpose E 128x128 tiles -> PSUM, cast-copy f16 -> SBUF
    PE mm2:     h += E_tile @ vh_tile  (accumulate over 64 column tiles)
    epilogue:   h/densum, ELU, DMA out (fp16)
"""

import os
import sys
import math

sys.path.insert(0, "/opt/trn_rl_repo")

import numpy as np

# The runner constructs a fresh jax.jit closure per call, so the identical
# HLO is re-lowered and re-compiled every invocation; the persistent cache
# turns those recompiles into disk hits.
try:
    import jax
    jax.config.update("jax_compilation_cache_dir", "/tmp/jax_comp_cache")
    jax.config.update("jax_persistent_cache_min_compile_time_secs", 0)
    jax.config.update("jax_persistent_cache_min_entry_size_bytes", -1)
except Exception:
    pass

import concourse.bass as bass
import concourse.mybir as mybir
import concourse.tile as tile
from concourse import bacc
from concourse import bass_utils
from concourse.masks import make_identity

F32 = mybir.dt.float32
F16 = mybir.dt.float16
U8 = mybir.dt.uint8
AF = mybir.ActivationFunctionType
ALU = mybir.AluOpType

NEG_BIG = 9.0e15  # matches reference NEG_INF magnitude


def build_kernel(n=8192, r_shard=1024, d=64, num_devices=8, use_cc=False,
                 quant_out=False, debug_blk=None):
    """Build the SPMD-uniform Bass program for one core.

    use_cc=False: khT [d,n] and vh [n,d] arrive replicated per core.
    use_cc=True:  only the per-core shards khT_own [d,R] / vh_own [R,d]
                  arrive; full khT/vh are assembled on-device with an
                  HBM-HBM AllGather across the 8 cores.
    """
    P = 128
    assert n % 1024 == 0 and r_shard % P == 0
    n_rblk = r_shard // P          # 128-row blocks per core
    n_ctile = n // P               # 128-col tiles
    wpk = n // 8                   # packed bytes per row

    nc = bacc.Bacc("TRN2", target_bir_lowering=False, debug=False,
                   enable_asserts=False, num_devices=num_devices)

    khT_own_d = nc.dram_tensor("khT_own", [d, r_shard], F16,
                               kind="ExternalInput").ap()
    adjp_d = nc.dram_tensor("adjp", [r_shard, wpk], U8,
                            kind="ExternalInput").ap()
    if quant_out:
        out_d = nc.dram_tensor("out", [r_shard, d], U8,
                               kind="ExternalOutput").ap()
        oscale_d = nc.dram_tensor("oscale", [r_shard, 1], F32,
                                  kind="ExternalOutput").ap()
    else:
        out_d = nc.dram_tensor("out", [r_shard, d], F16,
                               kind="ExternalOutput").ap()
    if debug_blk is not None:
        dbg_sm_d = nc.dram_tensor("dbg_sm", [P, 1024], F32,
                                  kind="ExternalOutput").ap()
        dbg_exp_d = nc.dram_tensor("dbg_exp", [P, 1024], F32,
                                   kind="ExternalOutput").ap()
        dbg_stats_d = nc.dram_tensor("dbg_stats", [P, 4], F32,
                                     kind="ExternalOutput").ap()
        dbg_et_d = nc.dram_tensor("dbg_et", [P, 512], F16,
                                  kind="ExternalOutput").ap()
        dbg_h_d = nc.dram_tensor("dbg_h", [P, d], F32,
                                 kind="ExternalOutput").ap()
    if use_cc:
        vh_own_d = nc.dram_tensor("vh_own", [r_shard, d], F16,
                                  kind="ExternalInput").ap()
    else:
        khT_d = nc.dram_tensor("khT", [d, n], F16, kind="ExternalInput").ap()
        vh_d = nc.dram_tensor("vh", [n, d], F16, kind="ExternalInput").ap()

    with tile.TileContext(nc) as tc:
        with tc.tile_pool(name="const", bufs=1) as cpool:
            ident = cpool.tile([P, P], F32, tag="ident")
            khT = cpool.tile([d, n], F16, tag="khT")
            khT_own = cpool.tile([d, r_shard], F16, tag="khTo")
            vh_sb = cpool.tile([P, n_ctile * d], F16, tag="vh")
            make_identity(nc, ident[:])

            nc.sync.dma_start(khT_own[:], khT_own_d[:, :])
            if use_cc:
                ncores = num_devices
                with tc.tile_pool(name="dram", bufs=1, space="DRAM") as dram:
                    kh_in = dram.tile([d, r_shard], F16, tag="khin")
                    kh_out = dram.tile([ncores * d, r_shard], F16, tag="khout")
                    vh_in = dram.tile([r_shard, d], F16, tag="vhin")
                    vh_out = dram.tile([ncores * r_shard, d], F16, tag="vhout")
                    nc.gpsimd.dma_start(kh_in[:], khT_own_d[:, :])
                    nc.gpsimd.dma_start(vh_in[:], vh_own_d[:, :])
                    groups = [list(range(ncores))]
                    nc.gpsimd.collective_compute(
                        "AllGather", ALU.bypass, replica_groups=groups,
                        ins=[kh_in.opt()], outs=[kh_out.opt()])
                    nc.gpsimd.collective_compute(
                        "AllGather", ALU.bypass, replica_groups=groups,
                        ins=[vh_in.opt()], outs=[vh_out.opt()])
                    for c in range(ncores):
                        nc.sync.dma_start(
                            khT[:, c * r_shard:(c + 1) * r_shard],
                            kh_out[c * d:(c + 1) * d, :])
                    for j in range(n_ctile):
                        nc.sync.dma_start(
                            vh_sb[:, j * d:(j + 1) * d],
                            vh_out[j * P:(j + 1) * P, :])
            else:
                for c in range(4):
                    w = n // 4
                    nc.sync.dma_start(khT[:, c * w:(c + 1) * w],
                                      khT_d[:, c * w:(c + 1) * w])
                for j in range(n_ctile):
                    nc.sync.dma_start(vh_sb[:, j * d:(j + 1) * d],
                                      vh_d[j * P:(j + 1) * P, :])

            # ---------------- main loop over 128-row blocks --------------
            with tc.tile_pool(name="adjp", bufs=3) as adjpp, \
                 tc.tile_pool(name="bits", bufs=2) as bitsp, \
                 tc.tile_pool(name="madj", bufs=2) as madjp, \
                 tc.tile_pool(name="sm", bufs=2) as smp, \
                 tc.tile_pool(name="et", bufs=3) as etp, \
                 tc.tile_pool(name="small", bufs=2) as smallp, \
                 tc.tile_pool(name="sp", bufs=2, space="PSUM") as spp, \
                 tc.tile_pool(name="tp", bufs=2, space="PSUM") as tpp, \
                 tc.tile_pool(name="hp", bufs=2, space="PSUM") as hpp:

                c_chunk = 1024
                for b in range(n_rblk):
                    r0 = b * P
                    sm = smp.tile([P, n], F32, tag="sm")

                    adjp_t = adjpp.tile([P, wpk], U8, tag="adjp")
                    nc.sync.dma_start(adjp_t[:], adjp_d[r0:r0 + P, :])
                    bits = bitsp.tile([P, n], U8, tag="bits")
                    for k in range(8):
                        nc.vector.tensor_scalar(
                            bits[:, k::8], adjp_t[:], 7 - k, 1,
                            ALU.logical_shift_right, ALU.bitwise_and)

                    for i in range(n // c_chunk):
                        c0 = i * c_chunk
                        madj_t = madjp.tile([P, c_chunk], F32, tag="madj")
                        nc.gpsimd.tensor_scalar(
                            madj_t[:], bits[:, c0:c0 + c_chunk],
                            NEG_BIG, NEG_BIG, ALU.mult, ALU.subtract)
                        sp = spp.tile([P, c_chunk], F32, tag="sp")
                        for half in range(c_chunk // 512):
                            h0 = half * 512
                            nc.tensor.matmul(
                                sp[:, h0:h0 + 512],
                                khT_own[:, r0:r0 + P],
                                khT[:, c0 + h0:c0 + h0 + 512],
                                start=True, stop=True)
                        nc.vector.tensor_tensor(
                            sm[:, c0:c0 + c_chunk], sp[:], madj_t[:], ALU.add)

                    if debug_blk == b:
                        nc.sync.dma_start(dbg_sm_d[:, :], sm[:, 0:1024])
                    # full-row max: E = exp(s - max) <= 1 keeps the f16
                    # E-tiles within range (sampled max can undershoot by
                    # >80 when the diagonal falls outside the sample window)
                    nsamp = n
                    chmax = smallp.tile([P, 1], F32, tag="chmax")
                    nc.vector.tensor_reduce(chmax[:], sm[:, 0:nsamp],
                                            mybir.AxisListType.X, ALU.max)
                    negmax = smallp.tile([P, 1], F32, tag="negmax")
                    nc.vector.tensor_scalar_mul(
                        negmax[:], chmax[:], -1.0 / math.sqrt(d))
                    densum = smallp.tile([P, 1], F32, tag="densum")
                    nc.scalar.activation(sm[:], sm[:], AF.Exp,
                                         bias=negmax[:],
                                         scale=1.0 / math.sqrt(d),
                                         accum_out=densum[:])
                    recipd = smallp.tile([P, 1], F32, tag="recipd")
                    nc.vector.reciprocal(recipd[:], densum[:])
                    if debug_blk == b:
                        nc.sync.dma_start(dbg_exp_d[:, :], sm[:, 0:1024])
                        stats = smallp.tile([P, 4], F32, tag="stats")
                        nc.vector.tensor_copy(stats[:, 0:1], chmax[:])
                        nc.vector.tensor_copy(stats[:, 1:2], negmax[:])
                        nc.vector.tensor_copy(stats[:, 2:3], densum[:])
                        nc.vector.tensor_copy(stats[:, 3:4], recipd[:])
                        nc.sync.dma_start(dbg_stats_d[:, :], stats[:])

                    hpt = hpp.tile([P, d], F32, tag="hp")
                    for j4 in range(n_ctile // 4):
                        tp = tpp.tile([P, 512], F32, tag="tp")
                        for q in range(4):
                            j = j4 * 4 + q
                            nc.tensor.transpose(
                                tp[:, q * P:(q + 1) * P],
                                sm[:, j * P:(j + 1) * P], ident[:])
                        et = etp.tile([P, 512], F16, tag="et")
                        if j4 % 2 == 1:
                            nc.vector.tensor_copy(et[:], tp[:])
                        else:
                            nc.scalar.copy(et[:], tp[:])
                        if debug_blk == b and j4 == 0:
                            nc.sync.dma_start(dbg_et_d[:, :], et[:])
                        for q in range(4):
                            j = j4 * 4 + q
                            nc.tensor.matmul(
                                hpt[:], et[:, q * P:(q + 1) * P],
                                vh_sb[:, j * d:(j + 1) * d],
                                start=(j == 0), stop=(j == n_ctile - 1))

                    h_sb = smallp.tile([P, d], F32, tag="h")
                    nc.vector.tensor_scalar(h_sb[:], hpt[:], recipd[:], None,
                                            ALU.mult)
                    if debug_blk == b:
                        nc.sync.dma_start(dbg_h_d[:, :], h_sb[:])
                    hneg = smallp.tile([P, d], F32, tag="hneg")
                    nc.vector.tensor_scalar(hneg[:], h_sb[:], 0.0, None, ALU.min)
                    hpos = smallp.tile([P, d], F32, tag="hpos")
                    nc.vector.tensor_scalar(hpos[:], h_sb[:], 0.0, None, ALU.max)
                    nc.scalar.activation(hneg[:], hneg[:], AF.Exp)
                    nc.vector.tensor_scalar(hneg[:], hneg[:], -1.0, None, ALU.add)
                    if quant_out:
                        # symmetric u8 quantization with per-row absmax scale:
                        # q = out*127/absmax + 128.5 (u8 cast), decoded on host
                        # as (q-128)*absmax/127; <=1 LSB = absmax/127 error
                        out_f = smallp.tile([P, d], F32, tag="outf")
                        nc.vector.tensor_tensor(out_f[:], hneg[:], hpos[:],
                                                ALU.add)
                        omax = smallp.tile([P, 1], F32, tag="omax")
                        nc.vector.tensor_reduce(omax[:], out_f[:],
                                                mybir.AxisListType.X, ALU.max)
                        omin = smallp.tile([P, 1], F32, tag="omin")
                        nc.vector.tensor_reduce(omin[:], out_f[:],
                                                mybir.AxisListType.X, ALU.min)
                        negmin = smallp.tile([P, 1], F32, tag="negmin")
                        nc.vector.tensor_scalar_mul(negmin[:], omin[:], -1.0)
                        oabs = smallp.tile([P, 1], F32, tag="oabs")
                        nc.vector.tensor_tensor(oabs[:], omax[:], negmin[:],
                                                ALU.max)
                        nc.vector.tensor_scalar_max(oabs[:], oabs[:], 1e-20)
                        orcp = smallp.tile([P, 1], F32, tag="orcp")
                        nc.vector.reciprocal(orcp[:], oabs[:])
                        qs = smallp.tile([P, 1], F32, tag="qs")
                        nc.vector.tensor_scalar_mul(qs[:], orcp[:], 127.0)
                        qt = smallp.tile([P, d], U8, tag="qt")
                        nc.gpsimd.tensor_scalar(qt[:], out_f[:], qs[:], 128.5,
                                                ALU.mult, ALU.add)
                        nc.sync.dma_start(out_d[r0:r0 + P, :], qt[:])
                        nc.sync.dma_start(oscale_d[r0:r0 + P, :], oabs[:])
                    else:
                        out_t = smallp.tile([P, d], F16, tag="outt")
                        nc.vector.tensor_tensor(out_t[:], hneg[:], hpos[:],
                                                ALU.add)
                        nc.sync.dma_start(out_d[r0:r0 + P, :], out_t[:])

    nc.compile()
    return nc


_NC_CACHE = {}

USE_CC = True
QUANT_OUT = False


def _get_nc(key=(8192, 1024, 64, 8, USE_CC, QUANT_OUT)):
    if key not in _NC_CACHE:
        _NC_CACHE[key] = build_kernel(*key)
    return _NC_CACHE[key]


def gather_out(res, ncores=8, quant_out=None):
    """Assemble the full [N, 64] f32 output from per-core results."""
    if quant_out is None:
        quant_out = QUANT_OUT
    out = np.concatenate([res.results[k]["out"] for k in range(ncores)],
                         axis=0).astype(np.float32)
    if quant_out:
        s = np.concatenate([res.results[k]["oscale"] for k in range(ncores)],
                           axis=0).astype(np.float32)
        out = (out - 128.0) * (s / 127.0)
    return out


def prep_inputs(input, adj, kW, vW, ncores=8, use_cc=USE_CC):
    """Host-side marshalling: project kh/vh, bit-pack adj, slice shards."""
    N, D = 8192, 64
    RS = N // ncores
    input = np.asarray(input, np.float32)
    kW = np.asarray(kW, np.float32)
    vW = np.asarray(vW, np.float32)
    kh = input @ kW                       # [N, D]
    vh = np.ascontiguousarray((input @ vW).astype(np.float16))  # [N, D]
    khT = np.ascontiguousarray(kh.T.astype(np.float16))  # [D, N]
    adjp = np.packbits(np.asarray(adj) > 0, axis=1)  # [N, N/8] u8

    in_maps = []
    for c in range(ncores):
        m = {
            "khT_own": np.ascontiguousarray(khT[:, c * RS:(c + 1) * RS]),
            "adjp": adjp[c * RS:(c + 1) * RS, :],
        }
        if use_cc:
            m["vh_own"] = vh[c * RS:(c + 1) * RS, :]
        else:
            m["khT"] = khT
            m["vh"] = vh
        in_maps.append(m)
    return in_maps


def kernel(input, adj, kW, vW):
    N, D, NCORES = 8192, 64, 8
    RS = N // NCORES
    assert np.asarray(input).shape == (N, 512) and np.asarray(adj).shape == (N, N)

    nc = _get_nc((N, RS, D, NCORES, USE_CC, QUANT_OUT))
    in_maps = prep_inputs(input, adj, kW, vW, NCORES, USE_CC)
    res = bass_utils.run_bass_kernel_spmd(nc, in_maps,
                                          core_ids=list(range(NCORES)))
    return gather_out(res, NCORES)


if __name__ == "__main__":
    rng = np.random.default_rng(0)
    x = rng.standard_normal((8192, 512), dtype=np.float32)
    a = (rng.random((8192, 8192)) < 0.5).astype(np.int32)
    kw = rng.uniform(-0.1, 0.1, (512, 64)).astype(np.float32)
    vw = rng.uniform(-0.1, 0.1, (512, 64)).astype(np.float32)
    o = kernel(input=x, adj=a, kW=kw, vW=vw)
    print(o.shape, o.dtype, np.abs(o).max())


# revision 20
# speedup vs baseline: 1.5773x; 1.0203x over previous
"""Trainium2 Bass kernel for masked (LSH/GNN) attention layer.

Computes, for full inputs (N=8192, F=512, D=64):
    kh = input @ kW; vh = input @ vW
    K  = (kh @ kh.T) / sqrt(D)
    scores = where(adj > 0, K, -9e15)
    att = softmax(scores, axis=1)
    out = elu(att @ vh)

Sharding: rows of the attention matrix across 8 NeuronCores (row-parallel).

The dominant cost in this environment is host->device staging over the
axon tunnel (~30-40 MB/s), so the input marshalling is laid out to move
as few bytes as possible per call (~11MB vs the baseline's ~400MB):
  - the tiny projections kh/vh ([N,64] each, 0.5% of the FLOPs) are
    computed on host from input/kW/vW, so the 16MB input matrix is never
    shipped (the baseline shipped it replicated to all 8 cores = 128MB);
  - kh/vh are shipped as fp16 *shards* (256KB/core each) and assembled
    on-device with an HBM-HBM AllGather across the 8 cores instead of
    being replicated; the N^2 attention math stays on-device;
  - adj is bit-packed on host (np.packbits) from 256MB int32 to 8MB of
    u8 (1MB/core), and unpacked on-device with shift/and tensor_scalar
    ops into the additive -9e15 mask;
  - output returns as fp16 (E<=1 and |h'| are well inside fp16 range;
    overall rel err ~1e-3 vs the 2e-2 gate);
  - the persistent jax compilation cache turns the runner's per-call
    re-jit of the identical HLO into a disk hit.
Device pipeline per 128-row block (row shard R=1024 per core):
    DMA:        adjp [128, N/8] u8 (packed adjacency rows)
    DVE:        unpack bits -> {0,1} u8 [128, N] (8 strided tensor_scalar)
    gpsimd:     madj = bits*9e15 - 9e15   in {0, -9e15}
    PE mm1:     S_raw = kh_own @ khT       (K=64 contraction, fp16)
    DVE:        sm = S_raw + madj ; full-row max (keeps E <= 1 for fp16)
    ACT:        E = exp((sm - rowmax)/sqrt(d)), accum_out -> densum
    PE:         transpose E 128x128 tiles -> PSUM, cast-copy f16 -> SBUF
    PE mm2:     h += E_tile @ vh_tile  (accumulate over 64 column tiles)
    epilogue:   h/densum, ELU, DMA out (fp16)
"""

import os
import sys
import math

sys.path.insert(0, "/opt/trn_rl_repo")

import numpy as np

# The runner constructs a fresh jax.jit closure per call, so the identical
# HLO is re-lowered and re-compiled every invocation; the persistent cache
# turns those recompiles into disk hits.
try:
    import jax
    jax.config.update("jax_compilation_cache_dir", "/tmp/jax_comp_cache")
    jax.config.update("jax_persistent_cache_min_compile_time_secs", 0)
    jax.config.update("jax_persistent_cache_min_entry_size_bytes", -1)
except Exception:
    pass

import concourse.bass as bass
import concourse.mybir as mybir
import concourse.tile as tile
from concourse import bacc
from concourse import bass_utils
from concourse.masks import make_identity

F32 = mybir.dt.float32
F16 = mybir.dt.float16
U8 = mybir.dt.uint8
AF = mybir.ActivationFunctionType
ALU = mybir.AluOpType

NEG_BIG = 9.0e15  # matches reference NEG_INF magnitude


def build_kernel(n=8192, r_shard=1024, d=64, num_devices=8, use_cc=False,
                 quant_out=False, debug_blk=None):
    """Build the SPMD-uniform Bass program for one core.

    use_cc=False: khT [d,n] and vh [n,d] arrive replicated per core.
    use_cc=True:  only the per-core shards khT_own [d,R] / vh_own [R,d]
                  arrive; full khT/vh are assembled on-device with an
                  HBM-HBM AllGather across the 8 cores.
    """
    P = 128
    assert n % 1024 == 0 and r_shard % P == 0
    n_rblk = r_shard // P          # 128-row blocks per core
    n_ctile = n // P               # 128-col tiles
    wpk = n // 8                   # packed bytes per row

    nc = bacc.Bacc("TRN2", target_bir_lowering=False, debug=False,
                   enable_asserts=False, num_devices=num_devices)

    khT_own_d = nc.dram_tensor("khT_own", [d, r_shard], F16,
                               kind="ExternalInput").ap()
    adjp_d = nc.dram_tensor("adjp", [r_shard, wpk], U8,
                            kind="ExternalInput").ap()
    if quant_out:
        # single packed output: 64 u8-quantized cols + per-row f32 scale
        # bitcast into the last 4 bytes (extra output tensors cost ~0.14s
        # per call in fetch RPC round trips, so everything rides in one)
        out_d = nc.dram_tensor("out", [r_shard, d + 4], U8,
                               kind="ExternalOutput").ap()
    else:
        out_d = nc.dram_tensor("out", [r_shard, d], F16,
                               kind="ExternalOutput").ap()
    if debug_blk is not None:
        dbg_sm_d = nc.dram_tensor("dbg_sm", [P, 1024], F32,
                                  kind="ExternalOutput").ap()
        dbg_exp_d = nc.dram_tensor("dbg_exp", [P, 1024], F32,
                                   kind="ExternalOutput").ap()
        dbg_stats_d = nc.dram_tensor("dbg_stats", [P, 4], F32,
                                     kind="ExternalOutput").ap()
        dbg_et_d = nc.dram_tensor("dbg_et", [P, 512], F16,
                                  kind="ExternalOutput").ap()
        dbg_h_d = nc.dram_tensor("dbg_h", [P, d], F32,
                                 kind="ExternalOutput").ap()
    if use_cc:
        vh_own_d = nc.dram_tensor("vh_own", [r_shard, d], F16,
                                  kind="ExternalInput").ap()
    else:
        khT_d = nc.dram_tensor("khT", [d, n], F16, kind="ExternalInput").ap()
        vh_d = nc.dram_tensor("vh", [n, d], F16, kind="ExternalInput").ap()

    with tile.TileContext(nc) as tc:
        with tc.tile_pool(name="const", bufs=1) as cpool:
            ident = cpool.tile([P, P], F32, tag="ident")
            khT = cpool.tile([d, n], F16, tag="khT")
            khT_own = cpool.tile([d, r_shard], F16, tag="khTo")
            vh_sb = cpool.tile([P, n_ctile * d], F16, tag="vh")
            make_identity(nc, ident[:])

            nc.sync.dma_start(khT_own[:], khT_own_d[:, :])
            if use_cc:
                ncores = num_devices
                with tc.tile_pool(name="dram", bufs=1, space="DRAM") as dram:
                    kh_in = dram.tile([d, r_shard], F16, tag="khin")
                    kh_out = dram.tile([ncores * d, r_shard], F16, tag="khout")
                    vh_in = dram.tile([r_shard, d], F16, tag="vhin")
                    vh_out = dram.tile([ncores * r_shard, d], F16, tag="vhout")
                    nc.gpsimd.dma_start(kh_in[:], khT_own_d[:, :])
                    nc.gpsimd.dma_start(vh_in[:], vh_own_d[:, :])
                    groups = [list(range(ncores))]
                    nc.gpsimd.collective_compute(
                        "AllGather", ALU.bypass, replica_groups=groups,
                        ins=[kh_in.opt()], outs=[kh_out.opt()])
                    nc.gpsimd.collective_compute(
                        "AllGather", ALU.bypass, replica_groups=groups,
                        ins=[vh_in.opt()], outs=[vh_out.opt()])
                    for c in range(ncores):
                        nc.sync.dma_start(
                            khT[:, c * r_shard:(c + 1) * r_shard],
                            kh_out[c * d:(c + 1) * d, :])
                    for j in range(n_ctile):
                        nc.sync.dma_start(
                            vh_sb[:, j * d:(j + 1) * d],
                            vh_out[j * P:(j + 1) * P, :])
            else:
                for c in range(4):
                    w = n // 4
                    nc.sync.dma_start(khT[:, c * w:(c + 1) * w],
                                      khT_d[:, c * w:(c + 1) * w])
                for j in range(n_ctile):
                    nc.sync.dma_start(vh_sb[:, j * d:(j + 1) * d],
                                      vh_d[j * P:(j + 1) * P, :])

            # ---------------- main loop over 128-row blocks --------------
            with tc.tile_pool(name="adjp", bufs=3) as adjpp, \
                 tc.tile_pool(name="bits", bufs=2) as bitsp, \
                 tc.tile_pool(name="madj", bufs=2) as madjp, \
                 tc.tile_pool(name="sm", bufs=2) as smp, \
                 tc.tile_pool(name="et", bufs=3) as etp, \
                 tc.tile_pool(name="small", bufs=2) as smallp, \
                 tc.tile_pool(name="sp", bufs=2, space="PSUM") as spp, \
                 tc.tile_pool(name="tp", bufs=2, space="PSUM") as tpp, \
                 tc.tile_pool(name="hp", bufs=2, space="PSUM") as hpp:

                c_chunk = 1024
                for b in range(n_rblk):
                    r0 = b * P
                    sm = smp.tile([P, n], F32, tag="sm")

                    adjp_t = adjpp.tile([P, wpk], U8, tag="adjp")
                    nc.sync.dma_start(adjp_t[:], adjp_d[r0:r0 + P, :])
                    bits = bitsp.tile([P, n], U8, tag="bits")
                    for k in range(8):
                        nc.vector.tensor_scalar(
                            bits[:, k::8], adjp_t[:], 7 - k, 1,
                            ALU.logical_shift_right, ALU.bitwise_and)

                    for i in range(n // c_chunk):
                        c0 = i * c_chunk
                        madj_t = madjp.tile([P, c_chunk], F32, tag="madj")
                        nc.gpsimd.tensor_scalar(
                            madj_t[:], bits[:, c0:c0 + c_chunk],
                            NEG_BIG, NEG_BIG, ALU.mult, ALU.subtract)
                        sp = spp.tile([P, c_chunk], F32, tag="sp")
                        for half in range(c_chunk // 512):
                            h0 = half * 512
                            nc.tensor.matmul(
                                sp[:, h0:h0 + 512],
                                khT_own[:, r0:r0 + P],
                                khT[:, c0 + h0:c0 + h0 + 512],
                                start=True, stop=True)
                        nc.vector.tensor_tensor(
                            sm[:, c0:c0 + c_chunk], sp[:], madj_t[:], ALU.add)

                    if debug_blk == b:
                        nc.sync.dma_start(dbg_sm_d[:, :], sm[:, 0:1024])
                    # full-row max: E = exp(s - max) <= 1 keeps the f16
                    # E-tiles within range (sampled max can undershoot by
                    # >80 when the diagonal falls outside the sample window)
                    nsamp = n
                    chmax = smallp.tile([P, 1], F32, tag="chmax")
                    nc.vector.tensor_reduce(chmax[:], sm[:, 0:nsamp],
                                            mybir.AxisListType.X, ALU.max)
                    negmax = smallp.tile([P, 1], F32, tag="negmax")
                    nc.vector.tensor_scalar_mul(
                        negmax[:], chmax[:], -1.0 / math.sqrt(d))
                    densum = smallp.tile([P, 1], F32, tag="densum")
                    nc.scalar.activation(sm[:], sm[:], AF.Exp,
                                         bias=negmax[:],
                                         scale=1.0 / math.sqrt(d),
                                         accum_out=densum[:])
                    recipd = smallp.tile([P, 1], F32, tag="recipd")
                    nc.vector.reciprocal(recipd[:], densum[:])
                    if debug_blk == b:
                        nc.sync.dma_start(dbg_exp_d[:, :], sm[:, 0:1024])
                        stats = smallp.tile([P, 4], F32, tag="stats")
                        nc.vector.tensor_copy(stats[:, 0:1], chmax[:])
                        nc.vector.tensor_copy(stats[:, 1:2], negmax[:])
                        nc.vector.tensor_copy(stats[:, 2:3], densum[:])
                        nc.vector.tensor_copy(stats[:, 3:4], recipd[:])
                        nc.sync.dma_start(dbg_stats_d[:, :], stats[:])

                    hpt = hpp.tile([P, d], F32, tag="hp")
                    for j4 in range(n_ctile // 4):
                        tp = tpp.tile([P, 512], F32, tag="tp")
                        for q in range(4):
                            j = j4 * 4 + q
                            nc.tensor.transpose(
                                tp[:, q * P:(q + 1) * P],
                                sm[:, j * P:(j + 1) * P], ident[:])
                        et = etp.tile([P, 512], F16, tag="et")
                        if j4 % 2 == 1:
                            nc.vector.tensor_copy(et[:], tp[:])
                        else:
                            nc.scalar.copy(et[:], tp[:])
                        if debug_blk == b and j4 == 0:
                            nc.sync.dma_start(dbg_et_d[:, :], et[:])
                        for q in range(4):
                            j = j4 * 4 + q
                            nc.tensor.matmul(
                                hpt[:], et[:, q * P:(q + 1) * P],
                                vh_sb[:, j * d:(j + 1) * d],
                                start=(j == 0), stop=(j == n_ctile - 1))

                    h_sb = smallp.tile([P, d], F32, tag="h")
                    nc.vector.tensor_scalar(h_sb[:], hpt[:], recipd[:], None,
                                            ALU.mult)
                    if debug_blk == b:
                        nc.sync.dma_start(dbg_h_d[:, :], h_sb[:])
                    hneg = smallp.tile([P, d], F32, tag="hneg")
                    nc.vector.tensor_scalar(hneg[:], h_sb[:], 0.0, None, ALU.min)
                    hpos = smallp.tile([P, d], F32, tag="hpos")
                    nc.vector.tensor_scalar(hpos[:], h_sb[:], 0.0, None, ALU.max)
                    nc.scalar.activation(hneg[:], hneg[:], AF.Exp)
                    nc.vector.tensor_scalar(hneg[:], hneg[:], -1.0, None, ALU.add)
                    if quant_out:
                        # symmetric u8 quantization with per-row absmax scale:
                        # q = out*127/absmax + 128 (u8 cast rounds-to-nearest),
                        # decoded on host as (q-128)*absmax/127; error <=0.5
                        # LSB = absmax/254 (<0.4% of the global max)
                        out_f = smallp.tile([P, d], F32, tag="outf")
                        nc.vector.tensor_tensor(out_f[:], hneg[:], hpos[:],
                                                ALU.add)
                        omax = smallp.tile([P, 1], F32, tag="omax")
                        nc.vector.tensor_reduce(omax[:], out_f[:],
                                                mybir.AxisListType.X, ALU.max)
                        omin = smallp.tile([P, 1], F32, tag="omin")
                        nc.vector.tensor_reduce(omin[:], out_f[:],
                                                mybir.AxisListType.X, ALU.min)
                        negmin = smallp.tile([P, 1], F32, tag="negmin")
                        nc.vector.tensor_scalar_mul(negmin[:], omin[:], -1.0)
                        oabs = smallp.tile([P, 1], F32, tag="oabs")
                        nc.vector.tensor_tensor(oabs[:], omax[:], negmin[:],
                                                ALU.max)
                        nc.vector.tensor_scalar_max(oabs[:], oabs[:], 1e-20)
                        orcp = smallp.tile([P, 1], F32, tag="orcp")
                        nc.vector.reciprocal(orcp[:], oabs[:])
                        qs = smallp.tile([P, 1], F32, tag="qs")
                        nc.vector.tensor_scalar_mul(qs[:], orcp[:], 127.0)
                        qt = smallp.tile([P, d], U8, tag="qt")
                        nc.gpsimd.tensor_scalar(qt[:], out_f[:], qs[:], 128.0,
                                                ALU.mult, ALU.add)
                        nc.sync.dma_start(out_d[r0:r0 + P, 0:d], qt[:])
                        nc.sync.dma_start(out_d[r0:r0 + P, d:d + 4],
                                          oabs[:].bitcast(U8))
                    else:
                        out_t = smallp.tile([P, d], F16, tag="outt")
                        nc.vector.tensor_tensor(out_t[:], hneg[:], hpos[:],
                                                ALU.add)
                        nc.sync.dma_start(out_d[r0:r0 + P, :], out_t[:])

    nc.compile()
    return nc


_NC_CACHE = {}

USE_CC = True
QUANT_OUT = True


def _get_nc(key=(8192, 1024, 64, 8, USE_CC, QUANT_OUT)):
    if key not in _NC_CACHE:
        _NC_CACHE[key] = build_kernel(*key)
    return _NC_CACHE[key]


def gather_out(res, ncores=8, quant_out=None):
    """Assemble the full [N, 64] f32 output from per-core results."""
    if quant_out is None:
        quant_out = QUANT_OUT
    raw = np.concatenate([res.results[k]["out"] for k in range(ncores)],
                         axis=0)
    if quant_out:
        q = raw[:, :64].astype(np.float32)
        s = np.ascontiguousarray(raw[:, 64:68]).view(np.float32)
        return (q - 128.0) * (s / 127.0)
    return raw.astype(np.float32)


def prep_inputs(input, adj, kW, vW, ncores=8, use_cc=USE_CC):
    """Host-side marshalling: project kh/vh, bit-pack adj, slice shards."""
    N, D = 8192, 64
    RS = N // ncores
    input = np.asarray(input, np.float32)
    kW = np.asarray(kW, np.float32)
    vW = np.asarray(vW, np.float32)
    kh = input @ kW                       # [N, D]
    vh = np.ascontiguousarray((input @ vW).astype(np.float16))  # [N, D]
    khT = np.ascontiguousarray(kh.T.astype(np.float16))  # [D, N]
    adjp = np.packbits(np.asarray(adj) > 0, axis=1)  # [N, N/8] u8

    in_maps = []
    for c in range(ncores):
        m = {
            "khT_own": np.ascontiguousarray(khT[:, c * RS:(c + 1) * RS]),
            "adjp": adjp[c * RS:(c + 1) * RS, :],
        }
        if use_cc:
            m["vh_own"] = vh[c * RS:(c + 1) * RS, :]
        else:
            m["khT"] = khT
            m["vh"] = vh
        in_maps.append(m)
    return in_maps


def kernel(input, adj, kW, vW):
    N, D, NCORES = 8192, 64, 8
    RS = N // NCORES
    assert np.asarray(input).shape == (N, 512) and np.asarray(adj).shape == (N, N)

    nc = _get_nc((N, RS, D, NCORES, USE_CC, QUANT_OUT))
    in_maps = prep_inputs(input, adj, kW, vW, NCORES, USE_CC)
    res = bass_utils.run_bass_kernel_spmd(nc, in_maps,
                                          core_ids=list(range(NCORES)))
    return gather_out(res, NCORES)


if __name__ == "__main__":
    rng = np.random.default_rng(0)
    x = rng.standard_normal((8192, 512), dtype=np.float32)
    a = (rng.random((8192, 8192)) < 0.5).astype(np.int32)
    kw = rng.uniform(-0.1, 0.1, (512, 64)).astype(np.float32)
    vw = rng.uniform(-0.1, 0.1, (512, 64)).astype(np.float32)
    o = kernel(input=x, adj=a, kW=kw, vW=vw)
    print(o.shape, o.dtype, np.abs(o).max())
